# revision 50
# baseline (speedup 1.0000x reference)
"""Trainium2 Bass kernel for AdditiveAttention (B=8, Lq=Lk=512, d_k=64).

Data-parallel over batch across 8 NeuronCores; each core computes one batch
element entirely on-chip.

Math: scores[q,k] = sum_d v_w[d] * tanh(qp[q,d] + kp[k,d]) with
qp = Q W_q^T, kp = K W_k^T.  tanh(a+b) is expanded in a sine series
    tanh(s) ~= sum_m c_m sin(w_m s)   (R=12 terms, max err ~1.1e-3 on |s|<=13)
so with sin(w(a+b)) = sin(wa)cos(wb) + cos(wa)sin(wb) the whole energy
tensor collapses into a matmul over a 2*R*64 contraction:
    scores^T = F_k^T-chunks (stationary) @ F_q (moving), accumulated in PSUM.
Features are built on-device:
  - U = (w_m/2pi)*proj + b  via augmented-weight fp32 matmuls (PE)
  - exact range reduction: r = round(U) via the fp32 magic-number trick
    (ACT Identity adds 1.5*2^23, DVE scalar_tensor_tensor computes r - U)
  - sin via ACT Sin on the centered fraction (scale=-2pi), bf16 out
  - q-side scaled by c_m*v_w[d] (DVE per-partition tensor_scalar)
Softmax runs in transposed layout (exp on ScalarE straight from PSUM; row
sums via ones-matmul; reciprocal broadcast by PE outer product), attn is
written transposed (host returns a transposed view), and out = attn @ V uses
the exp^T tiles directly with a ones-column appended to V to recover the
softmax denominators in q-layout.

Self-contained: hardcodes shapes; host-side work is layout + tiny constants.
"""
import numpy as np
import ml_dtypes

import concourse.bass as bass
import concourse.tile as tile
from concourse import bacc, mybir
from concourse.bass_utils import run_bass_kernel_spmd

B = 8
LQ = 512
LK = 512
D = 64
N_CORES = 8

R = 12          # number of sine terms
FIT_S = 13.0    # fit domain half-width
FIT_W = 4.5     # max angular frequency (max err ~1.1e-3 on [-13, 13])
MAGIC = float(np.float32(1.5 * 2 ** 23))

FP32 = mybir.dt.float32
F32R = mybir.dt.float32r
BF16 = mybir.dt.bfloat16

_CACHE: dict = {}


def _fit_sines():
    w = (np.arange(R) + 0.5) * FIT_W / R
    s = np.linspace(-FIT_S, FIT_S, 8001)
    A = np.sin(np.outer(s, w))
    y = np.tanh(s)
    c = np.linalg.solve(A.T @ A + 1e-8 * np.eye(R), A.T @ y)
    return w.astype(np.float64), c.astype(np.float64)


FREQS, COEFS = _fit_sines()


def _build():
    nc = bacc.Bacc("TRN2", target_bir_lowering=False, debug=False,
                   num_devices=N_CORES)

    qaug = nc.declare_dram_parameter("qaug", [D + 1, LQ], F32R, isOutput=False)
    kaug = nc.declare_dram_parameter("kaug", [D + 1, LK], F32R, isOutput=False)
    v = nc.declare_dram_parameter("v", [LK, D], BF16, isOutput=False)
    wqa = nc.declare_dram_parameter("wqa", [D + 1, R * 128], F32R, isOutput=False)
    wka = nc.declare_dram_parameter("wka", [D + 1, R * 128], F32R, isOutput=False)
    coef = nc.declare_dram_parameter("coef", [128, R], FP32, isOutput=False)
    attnT_out = nc.declare_dram_parameter("attnT", [LK, LQ], BF16, isOutput=True)
    out_out = nc.declare_dram_parameter("out", [LQ, D], FP32, isOutput=True)

    with tile.TileContext(nc) as tc:
        _program(nc, tc, qaug, kaug, v, wqa, wka, coef, attnT_out, out_out)

    nc.compile()
    return nc


def _program(nc, tc, qaug, kaug, v, wqa, wka, coef, attnT_out, out_out):
    T1_ALT = True    # alternate the round-add between DVE and ScalarE
    NH = 1           # no q-split (PSUM accumulation is per-element via memset+start=False)
    COEF_GP = True   # coefficient scaling on GpSimd
    TAIL_GP = False
    HW = LQ // NH
    QTILES = LQ // 128  # 4
    KCH = LK // 128     # 4
    SINF = mybir.ActivationFunctionType.Sin
    IDF = mybir.ActivationFunctionType.Identity
    EXPF = mybir.ActivationFunctionType.Exp

    with (
        tc.tile_pool(name="const", bufs=1) as const_pool,
        tc.tile_pool(name="work", bufs=1) as work_pool,
        tc.tile_pool(name="scT_ps", bufs=1, space="PSUM") as scT_pool,
    ):
        # ---- constants / inputs ----
        qaug_sb = const_pool.tile([D + 1, LQ], F32R)
        nc.sync.dma_start(qaug_sb[:], qaug[:])
        kaug_sb = const_pool.tile([D + 1, LK], F32R)
        nc.sync.dma_start(kaug_sb[:], kaug[:])
        wqa_sb = const_pool.tile([D + 1, R * 128], F32R)
        wka_sb = const_pool.tile([D + 1, R * 128], F32R)
        warm_sb = const_pool.tile([128, 128], BF16)
        nc.vector.memset(warm_sb[:, 0:16], 0.0)
        with tc.tile_pool(name="warm_ps", bufs=1, space="PSUM") as warm_pool:
            warm_ps = warm_pool.tile([128, 128], FP32)
            for _ in range(10):
                nc.tensor.matmul(warm_ps[:], warm_sb[:], warm_sb[:],
                                 start=True, stop=True)
        first = 2 * 128
        nc.sync.dma_start(wqa_sb[:, 0:first], wqa[:, 0:first])
        nc.sync.dma_start(wka_sb[:, 0:first], wka[:, 0:first])
        nc.sync.dma_start(wqa_sb[:, first:R * 128], wqa[:, first:R * 128])
        nc.sync.dma_start(wka_sb[:, first:R * 128], wka[:, first:R * 128])
        coef_sb = const_pool.tile([128, R], FP32)
        nc.sync.dma_start(coef_sb[:], coef[:])
        magic = const_pool.tile([128, 1], FP32)
        nc.vector.memset(magic[:], MAGIC)
        ones_col = const_pool.tile([128, 1], BF16)
        nc.vector.memset(ones_col[:], 1.0)
        ones_row = const_pool.tile([1, 128], BF16)
        nc.vector.memset(ones_row[:], 1.0)
        v1_tiles = const_pool.tile([128, KCH * (D + 1)], BF16)
        v_r = v.rearrange("(c p) d -> p c d", p=128)
        v1_r = v1_tiles.rearrange("p (c e) -> p c e", c=KCH)
        nc.sync.dma_start(v1_r[:, :, 0:D], v_r[:])
        for c in range(KCH):
            nc.vector.memset(v1_tiles[:, c * (D + 1) + D: (c + 1) * (D + 1)], 1.0)

        # ---- phase 1+2: features and score matmuls, pipelined over i ----
        # interleaved layout: block i = [q-side freq i (512) | k-side freq i (512)]
        w_all = work_pool.tile([128, R * 1024], FP32)  # centered -frac(U)
        f_all = work_pool.tile([128, R * 1024], BF16)  # sin features (raw)
        fq = work_pool.tile([128, R * LQ], BF16)       # coef-scaled q features
        scT = [scT_pool.tile([128, LQ], FP32, name=f"scT{c}") for c in range(KCH)]

        # software-pipelined over i: stage deps are >=1 iteration old so no
        # engine queue ever stalls on an in-flight producer.
        with tc.tile_pool(name="u_ps", bufs=2, space="PSUM") as u_pool:
          u_tiles = {}
          t1_tiles = {}
          for it in range(R + 2):
            if it < R:
                i = it
                u_ps = u_pool.tile([128, 1024], FP32, name="u_ps", tag="u_ps")
                u_tiles[i] = u_ps
                nc.tensor.matmul(u_ps[:, 0:512], wqa_sb[:, i * 128:(i + 1) * 128],
                                 qaug_sb[:], start=True, stop=True)
                nc.tensor.matmul(u_ps[:, 512:1024], wka_sb[:, i * 128:(i + 1) * 128],
                                 kaug_sb[:], start=True, stop=True)
                t1 = work_pool.tile([128, 1024], FP32, name="t1", tag="t1", bufs=4)
                t1_tiles[i] = t1
                if T1_ALT and i % 2 == 0:
                    nc.vector.tensor_scalar(t1[:], u_ps[:], MAGIC, None,
                                            mybir.AluOpType.add)
                else:
                    nc.scalar.activation(t1[:], u_ps[:], IDF, bias=magic[:, 0:1])
            if 1 <= it <= R:
                i = it - 1
                nc.vector.scalar_tensor_tensor(
                    w_all[:, i * 1024:(i + 1) * 1024], t1_tiles.pop(i)[:], -MAGIC,
                    u_tiles.pop(i)[:],
                    mybir.AluOpType.add, mybir.AluOpType.subtract)
            if 2 <= it:
                i = it - 2
                lo, hi = i * 1024, (i + 1) * 1024
                nc.scalar.activation(f_all[:, lo:hi], w_all[:, lo:hi], SINF,
                                     scale=float(-2 * np.pi))
                _coef_eng = nc.gpsimd if COEF_GP else nc.vector
                _coef_eng.tensor_scalar(
                    fq[:, i * 512:(i + 1) * 512],
                    f_all[:, i * 1024: i * 1024 + 512],
                    coef_sb[:, i:i + 1], None, mybir.AluOpType.mult)
                for c in range(KCH):
                    for h in range(NH):
                        nc.tensor.matmul(
                            scT[c][:, h * HW: (h + 1) * HW],
                            f_all[:, i * 1024 + 512 + c * 128:
                                  i * 1024 + 512 + (c + 1) * 128],
                            fq[:, i * 512 + h * HW: i * 512 + (h + 1) * HW],
                            start=(i == 0), stop=(i == R - 1))

        # ---- phase 3+4 per q-half: softmax in T layout, attnT out, AV ----
        expT = work_pool.tile([128, KCH * LQ], BF16)
        with (
            tc.tile_pool(name="sm_ps", bufs=1, space="PSUM") as sm_ps,
            tc.tile_pool(name="av_ps", bufs=2, space="PSUM") as av_ps_pool,
        ):
            for h in range(NH):
                hlo = h * HW
                for c in range(KCH):
                    nc.scalar.activation(
                        expT[:, c * LQ + hlo: c * LQ + hlo + HW],
                        scT[c][:, hlo: hlo + HW], EXPF)
                sums_row = sm_ps.tile([1, HW], FP32, name="sums_row", tag="sums")
                for c in range(KCH):
                    nc.tensor.matmul(sums_row[:], ones_col[:],
                                     expT[:, c * LQ + hlo: c * LQ + hlo + HW],
                                     start=(c == 0), stop=(c == KCH - 1))
                recip_row = work_pool.tile([1, HW], BF16, name="recip_row",
                                           tag="recip_row", bufs=NH)
                with nc.allow_low_precision(reason="bf16 softmax recip bcast"):
                    nc.vector.reciprocal(recip_row[:], sums_row[:])
                rbc_ps = sm_ps.tile([128, HW], FP32, name="rbc_ps", tag="rbc")
                nc.tensor.matmul(rbc_ps[:], ones_row[:], recip_row[:],
                                 start=True, stop=True)
                rbc_sb = work_pool.tile([128, HW], BF16, name="rbc_sb",
                                        tag="rbc_sb", bufs=NH)
                nc.scalar.activation(rbc_sb[:], rbc_ps[:], IDF)
                for c in range(KCH):
                    a_t = work_pool.tile([128, HW], BF16, name="a_t", tag="a_t",
                                         bufs=4)
                    eng = nc.gpsimd if (TAIL_GP and c % 2 == 1) else nc.vector
                    eng.tensor_tensor(a_t[:],
                                      expT[:, c * LQ + hlo: c * LQ + hlo + HW],
                                      rbc_sb[:], mybir.AluOpType.mult)
                    nc.sync.dma_start(attnT_out[c * 128:(c + 1) * 128, hlo:hlo + HW],
                                      a_t[:])
                o_all = work_pool.tile([128, (HW // 128) * D], FP32,
                                       name="o_all", tag="o_all", bufs=NH)
                for t in range(HW // 128):
                    qlo = hlo + t * 128
                    av_ps = av_ps_pool.tile([128, D + 1], FP32, name="av_ps")
                    for c in range(KCH):
                        nc.tensor.matmul(av_ps[:],
                                         expT[:, c * LQ + qlo: c * LQ + qlo + 128],
                                         v1_tiles[:, c * (D + 1):(c + 1) * (D + 1)],
                                         start=(c == 0), stop=(c == KCH - 1))
                    recip_q = work_pool.tile([128, 1], FP32, name="recip_q",
                                             tag="recip_q", bufs=2)
                    nc.vector.reciprocal(recip_q[:], av_ps[:, D:D + 1])
                    nc.vector.tensor_scalar(o_all[:, t * D:(t + 1) * D],
                                            av_ps[:, 0:D], recip_q[:, 0:1],
                                            None, mybir.AluOpType.mult)
                out_r = out_out.rearrange("(t p) d -> p t d", p=128)
                o_all_r = o_all.rearrange("p (t d) -> p t d", d=D)
                nc.sync.dma_start(out_r[:], o_all_r[:])


def _prep_inputs(Q, K, V, W_q, W_k, v_w):
    Q = np.asarray(Q, dtype=np.float32)
    K = np.asarray(K, dtype=np.float32)
    V = np.asarray(V, dtype=np.float32)
    W_q = np.asarray(W_q, dtype=np.float32)
    W_k = np.asarray(W_k, dtype=np.float32)
    v_w = np.asarray(v_w, dtype=np.float32)

    f = (FREQS / (2 * np.pi)).astype(np.float64)
    # wqa tile i, col p: rows 0-63 = f_i * W_q[p%64, :]; row 64 = b (0 sin / .25 cos)
    wqa = np.zeros((D + 1, R * 128), dtype=np.float32)
    wka = np.zeros((D + 1, R * 128), dtype=np.float32)
    for i in range(R):
        blk_q = np.concatenate([f[i] * W_q.T, f[i] * W_q.T], axis=1)  # [64, 128]
        blk_k = np.concatenate([f[i] * W_k.T, f[i] * W_k.T], axis=1)
        wqa[:D, i * 128:(i + 1) * 128] = blk_q
        wka[:D, i * 128:(i + 1) * 128] = blk_k
        # q side: rows 0-63 sin (b=0), 64-127 cos (b=0.25)
        wqa[D, i * 128 + 64: (i + 1) * 128] = 0.25
        # k side: rows 0-63 cos (b=0.25), 64-127 sin (b=0)
        wka[D, i * 128: i * 128 + 64] = 0.25
    coef = np.zeros((128, R), dtype=np.float32)
    for i in range(R):
        cw = (COEFS[i] * v_w.astype(np.float64)).astype(np.float32)
        coef[:D, i] = cw
        coef[D:, i] = cw

    ones = np.ones((1, LQ), dtype=np.float32)
    in_maps = []
    for b in range(B):
        in_maps.append({
            "qaug": np.concatenate([Q[b].T, ones], axis=0),
            "kaug": np.concatenate([K[b].T, ones], axis=0),
            "v": np.ascontiguousarray(V[b]).astype(ml_dtypes.bfloat16),
            "wqa": wqa, "wka": wka, "coef": coef,
        })
    return in_maps


def _get_nc():
    if "nc" not in _CACHE:
        _CACHE["nc"] = _build()
    return _CACHE["nc"]


def run(Q, K, V, W_q, W_k, v_w, trace=False, **spmd_kwargs):
    nc = _get_nc()
    in_maps = _prep_inputs(Q, K, V, W_q, W_k, v_w)
    res = run_bass_kernel_spmd(nc, in_maps, core_ids=list(range(N_CORES)),
                               trace=trace, **spmd_kwargs)
    out = np.stack([np.asarray(r["out"], dtype=np.float32) for r in res.results])
    attn = np.stack([np.asarray(r["attnT"].T, dtype=np.float32)
                     for r in res.results])
    return (out, attn), res


def kernel(Q, K, V, W_q, W_k, v_w):
    (out, attn), _ = run(Q, K, V, W_q, W_k, v_w)
    return (out, attn)


# revision 51
# speedup vs baseline: 1.0061x; 1.0061x over previous
"""Trainium2 Bass kernel for AdditiveAttention (B=8, Lq=Lk=512, d_k=64).

Data-parallel over batch across 8 NeuronCores; each core computes one batch
element entirely on-chip.

Math: scores[q,k] = sum_d v_w[d] * tanh(qp[q,d] + kp[k,d]) with
qp = Q W_q^T, kp = K W_k^T.  tanh(a+b) is expanded in a sine series
    tanh(s) ~= sum_m c_m sin(w_m s)   (R=12 terms, max err ~1.1e-3 on |s|<=13)
so with sin(w(a+b)) = sin(wa)cos(wb) + cos(wa)sin(wb) the whole energy
tensor collapses into a matmul over a 2*R*64 contraction:
    scores^T = F_k^T-chunks (stationary) @ F_q (moving), accumulated in PSUM.
Features are built on-device:
  - U = (w_m/2pi)*proj + b  via augmented-weight fp32 matmuls (PE)
  - exact range reduction: r = round(U) via the fp32 magic-number trick
    (ACT Identity adds 1.5*2^23, DVE scalar_tensor_tensor computes r - U)
  - sin via ACT Sin on the centered fraction (scale=-2pi), bf16 out
  - q-side scaled by c_m*v_w[d] (DVE per-partition tensor_scalar)
Softmax runs in transposed layout (exp on ScalarE straight from PSUM; row
sums via ones-matmul; reciprocal broadcast by PE outer product), attn is
written transposed (host returns a transposed view), and out = attn @ V uses
the exp^T tiles directly with a ones-column appended to V to recover the
softmax denominators in q-layout.

Self-contained: hardcodes shapes; host-side work is layout + tiny constants.
"""
import numpy as np
import ml_dtypes

import concourse.bass as bass
import concourse.tile as tile
from concourse import bacc, mybir
from concourse.bass_utils import run_bass_kernel_spmd

B = 8
LQ = 512
LK = 512
D = 64
N_CORES = 8

R = 12          # number of sine terms
FIT_S = 13.0    # fit domain half-width
FIT_W = 4.5     # max angular frequency (max err ~1.1e-3 on [-13, 13])
MAGIC = float(np.float32(1.5 * 2 ** 23))

FP32 = mybir.dt.float32
F32R = mybir.dt.float32r
BF16 = mybir.dt.bfloat16

_CACHE: dict = {}


def _fit_sines():
    w = (np.arange(R) + 0.5) * FIT_W / R
    s = np.linspace(-FIT_S, FIT_S, 8001)
    A = np.sin(np.outer(s, w))
    y = np.tanh(s)
    c = np.linalg.solve(A.T @ A + 1e-8 * np.eye(R), A.T @ y)
    return w.astype(np.float64), c.astype(np.float64)


FREQS, COEFS = _fit_sines()


def _build():
    nc = bacc.Bacc("TRN2", target_bir_lowering=False, debug=False,
                   num_devices=N_CORES)

    qaug = nc.declare_dram_parameter("qaug", [D + 1, LQ], F32R, isOutput=False)
    kaug = nc.declare_dram_parameter("kaug", [D + 1, LK], F32R, isOutput=False)
    v = nc.declare_dram_parameter("v", [LK, D], BF16, isOutput=False)
    wqa = nc.declare_dram_parameter("wqa", [D + 1, R * 128], F32R, isOutput=False)
    wka = nc.declare_dram_parameter("wka", [D + 1, R * 128], F32R, isOutput=False)
    coef = nc.declare_dram_parameter("coef", [128, R], FP32, isOutput=False)
    attnT_out = nc.declare_dram_parameter("attnT", [LK, LQ], BF16, isOutput=True)
    out_out = nc.declare_dram_parameter("out", [LQ, D], FP32, isOutput=True)

    with tile.TileContext(nc) as tc:
        _program(nc, tc, qaug, kaug, v, wqa, wka, coef, attnT_out, out_out)

    nc.compile()
    return nc


def _program(nc, tc, qaug, kaug, v, wqa, wka, coef, attnT_out, out_out):
    T1_ALT = True    # alternate the round-add between DVE and ScalarE
    NH = 1           # no q-split (PSUM accumulation is per-element via memset+start=False)
    COEF_GP = True   # coefficient scaling on GpSimd
    TAIL_GP = False
    HW = LQ // NH
    QTILES = LQ // 128  # 4
    KCH = LK // 128     # 4
    SINF = mybir.ActivationFunctionType.Sin
    IDF = mybir.ActivationFunctionType.Identity
    EXPF = mybir.ActivationFunctionType.Exp

    with (
        tc.tile_pool(name="const", bufs=1) as const_pool,
        tc.tile_pool(name="work", bufs=1) as work_pool,
        tc.tile_pool(name="scT_ps", bufs=1, space="PSUM") as scT_pool,
    ):
        # ---- constants / inputs ----
        qaug_sb = const_pool.tile([D + 1, LQ], F32R)
        nc.sync.dma_start(qaug_sb[:], qaug[:])
        kaug_sb = const_pool.tile([D + 1, LK], F32R)
        nc.sync.dma_start(kaug_sb[:], kaug[:])
        wqa_sb = const_pool.tile([D + 1, R * 128], F32R)
        wka_sb = const_pool.tile([D + 1, R * 128], F32R)
        warm_sb = const_pool.tile([128, 128], BF16)
        nc.vector.memset(warm_sb[:, 0:16], 0.0)
        with tc.tile_pool(name="warm_ps", bufs=1, space="PSUM") as warm_pool:
            warm_ps = warm_pool.tile([128, 128], FP32)
            for _ in range(10):
                nc.tensor.matmul(warm_ps[:], warm_sb[:], warm_sb[:],
                                 start=True, stop=True)
        first = 2 * 128
        nc.sync.dma_start(wqa_sb[:, 0:first], wqa[:, 0:first])
        nc.sync.dma_start(wka_sb[:, 0:first], wka[:, 0:first])
        nc.sync.dma_start(wqa_sb[:, first:R * 128], wqa[:, first:R * 128])
        nc.sync.dma_start(wka_sb[:, first:R * 128], wka[:, first:R * 128])
        coef_sb = const_pool.tile([128, R], FP32)
        nc.sync.dma_start(coef_sb[:], coef[:])
        magic = const_pool.tile([128, 1], FP32)
        nc.vector.memset(magic[:], MAGIC)
        ones_col = const_pool.tile([128, 1], BF16)
        nc.vector.memset(ones_col[:], 1.0)
        ones_row = const_pool.tile([1, 128], BF16)
        nc.vector.memset(ones_row[:], 1.0)
        v1_tiles = const_pool.tile([128, KCH * (D + 1)], BF16)
        v_r = v.rearrange("(c p) d -> p c d", p=128)
        v1_r = v1_tiles.rearrange("p (c e) -> p c e", c=KCH)
        nc.sync.dma_start(v1_r[:, :, 0:D], v_r[:])
        for c in range(KCH):
            nc.vector.memset(v1_tiles[:, c * (D + 1) + D: (c + 1) * (D + 1)], 1.0)

        # ---- phase 1+2: features and score matmuls, pipelined over i ----
        # interleaved layout: block i = [q-side freq i (512) | k-side freq i (512)]
        w_all = work_pool.tile([128, R * 1024], FP32)  # centered -frac(U)
        f_all = work_pool.tile([128, R * 1024], BF16)  # sin features (raw)
        fq = work_pool.tile([128, R * LQ], BF16)       # coef-scaled q features
        scT = [scT_pool.tile([128, LQ], FP32, name=f"scT{c}") for c in range(KCH)]

        # software-pipelined over i: stage deps are >=1 iteration old so no
        # engine queue ever stalls on an in-flight producer.
        with tc.tile_pool(name="u_ps", bufs=2, space="PSUM") as u_pool:
          u_tiles = {}
          t1_tiles = {}
          for it in range(R + 2):
            if it < R:
                i = it
                u_ps = u_pool.tile([128, 1024], FP32, name="u_ps", tag="u_ps")
                u_tiles[i] = u_ps
                nc.tensor.matmul(u_ps[:, 0:512], wqa_sb[:, i * 128:(i + 1) * 128],
                                 qaug_sb[:], start=True, stop=True)
                nc.tensor.matmul(u_ps[:, 512:1024], wka_sb[:, i * 128:(i + 1) * 128],
                                 kaug_sb[:], start=True, stop=True)
                if i > 0:
                    # i=0: |2pi*u| < pi for the lowest frequency -- no range
                    # reduction needed; sin reads the U psum tile directly.
                    t1 = work_pool.tile([128, 1024], FP32, name="t1", tag="t1",
                                        bufs=4)
                    t1_tiles[i] = t1
                    if T1_ALT and i % 2 == 0:
                        nc.vector.tensor_scalar(t1[:], u_ps[:], MAGIC, None,
                                                mybir.AluOpType.add)
                    else:
                        nc.scalar.activation(t1[:], u_ps[:], IDF,
                                             bias=magic[:, 0:1])
            if 1 <= it <= R:
                i = it - 1
                if i > 0:
                    nc.vector.scalar_tensor_tensor(
                        w_all[:, i * 1024:(i + 1) * 1024], t1_tiles.pop(i)[:],
                        -MAGIC, u_tiles.pop(i)[:],
                        mybir.AluOpType.add, mybir.AluOpType.subtract)
            if 2 <= it:
                i = it - 2
                lo, hi = i * 1024, (i + 1) * 1024
                if i == 0:
                    nc.scalar.activation(f_all[:, lo:hi], u_tiles.pop(0)[:], SINF,
                                         scale=float(2 * np.pi))
                else:
                    nc.scalar.activation(f_all[:, lo:hi], w_all[:, lo:hi], SINF,
                                         scale=float(-2 * np.pi))
                _coef_eng = nc.gpsimd if COEF_GP else nc.vector
                _coef_eng.tensor_scalar(
                    fq[:, i * 512:(i + 1) * 512],
                    f_all[:, i * 1024: i * 1024 + 512],
                    coef_sb[:, i:i + 1], None, mybir.AluOpType.mult)
                for c in range(KCH):
                    for h in range(NH):
                        nc.tensor.matmul(
                            scT[c][:, h * HW: (h + 1) * HW],
                            f_all[:, i * 1024 + 512 + c * 128:
                                  i * 1024 + 512 + (c + 1) * 128],
                            fq[:, i * 512 + h * HW: i * 512 + (h + 1) * HW],
                            start=(i == 0), stop=(i == R - 1))

        # ---- phase 3+4 per q-half: softmax in T layout, attnT out, AV ----
        expT = work_pool.tile([128, KCH * LQ], BF16)
        with (
            tc.tile_pool(name="sm_ps", bufs=1, space="PSUM") as sm_ps,
            tc.tile_pool(name="av_ps", bufs=2, space="PSUM") as av_ps_pool,
        ):
            for h in range(NH):
                hlo = h * HW
                for c in range(KCH):
                    nc.scalar.activation(
                        expT[:, c * LQ + hlo: c * LQ + hlo + HW],
                        scT[c][:, hlo: hlo + HW], EXPF)
                sums_row = sm_ps.tile([1, HW], FP32, name="sums_row", tag="sums")
                for c in range(KCH):
                    nc.tensor.matmul(sums_row[:], ones_col[:],
                                     expT[:, c * LQ + hlo: c * LQ + hlo + HW],
                                     start=(c == 0), stop=(c == KCH - 1))
                recip_row = work_pool.tile([1, HW], BF16, name="recip_row",
                                           tag="recip_row", bufs=NH)
                with nc.allow_low_precision(reason="bf16 softmax recip bcast"):
                    nc.vector.reciprocal(recip_row[:], sums_row[:])
                rbc_ps = sm_ps.tile([128, HW], FP32, name="rbc_ps", tag="rbc")
                nc.tensor.matmul(rbc_ps[:], ones_row[:], recip_row[:],
                                 start=True, stop=True)
                rbc_sb = work_pool.tile([128, HW], BF16, name="rbc_sb",
                                        tag="rbc_sb", bufs=NH)
                nc.scalar.activation(rbc_sb[:], rbc_ps[:], IDF)
                for c in range(KCH):
                    a_t = work_pool.tile([128, HW], BF16, name="a_t", tag="a_t",
                                         bufs=4)
                    eng = nc.gpsimd if (TAIL_GP and c % 2 == 1) else nc.vector
                    eng.tensor_tensor(a_t[:],
                                      expT[:, c * LQ + hlo: c * LQ + hlo + HW],
                                      rbc_sb[:], mybir.AluOpType.mult)
                    nc.sync.dma_start(attnT_out[c * 128:(c + 1) * 128, hlo:hlo + HW],
                                      a_t[:])
                o_all = work_pool.tile([128, (HW // 128) * D], FP32,
                                       name="o_all", tag="o_all", bufs=NH)
                for t in range(HW // 128):
                    qlo = hlo + t * 128
                    av_ps = av_ps_pool.tile([128, D + 1], FP32, name="av_ps")
                    for c in range(KCH):
                        nc.tensor.matmul(av_ps[:],
                                         expT[:, c * LQ + qlo: c * LQ + qlo + 128],
                                         v1_tiles[:, c * (D + 1):(c + 1) * (D + 1)],
                                         start=(c == 0), stop=(c == KCH - 1))
                    recip_q = work_pool.tile([128, 1], FP32, name="recip_q",
                                             tag="recip_q", bufs=2)
                    nc.vector.reciprocal(recip_q[:], av_ps[:, D:D + 1])
                    nc.vector.tensor_scalar(o_all[:, t * D:(t + 1) * D],
                                            av_ps[:, 0:D], recip_q[:, 0:1],
                                            None, mybir.AluOpType.mult)
                out_r = out_out.rearrange("(t p) d -> p t d", p=128)
                o_all_r = o_all.rearrange("p (t d) -> p t d", d=D)
                nc.sync.dma_start(out_r[:], o_all_r[:])


def _prep_inputs(Q, K, V, W_q, W_k, v_w):
    Q = np.asarray(Q, dtype=np.float32)
    K = np.asarray(K, dtype=np.float32)
    V = np.asarray(V, dtype=np.float32)
    W_q = np.asarray(W_q, dtype=np.float32)
    W_k = np.asarray(W_k, dtype=np.float32)
    v_w = np.asarray(v_w, dtype=np.float32)

    f = (FREQS / (2 * np.pi)).astype(np.float64)
    # wqa tile i, col p: rows 0-63 = f_i * W_q[p%64, :]; row 64 = b (0 sin / .25 cos)
    wqa = np.zeros((D + 1, R * 128), dtype=np.float32)
    wka = np.zeros((D + 1, R * 128), dtype=np.float32)
    for i in range(R):
        blk_q = np.concatenate([f[i] * W_q.T, f[i] * W_q.T], axis=1)  # [64, 128]
        blk_k = np.concatenate([f[i] * W_k.T, f[i] * W_k.T], axis=1)
        wqa[:D, i * 128:(i + 1) * 128] = blk_q
        wka[:D, i * 128:(i + 1) * 128] = blk_k
        # q side: rows 0-63 sin (b=0), 64-127 cos (b=0.25)
        wqa[D, i * 128 + 64: (i + 1) * 128] = 0.25
        # k side: rows 0-63 cos (b=0.25), 64-127 sin (b=0)
        wka[D, i * 128: i * 128 + 64] = 0.25
    coef = np.zeros((128, R), dtype=np.float32)
    for i in range(R):
        cw = (COEFS[i] * v_w.astype(np.float64)).astype(np.float32)
        coef[:D, i] = cw
        coef[D:, i] = cw

    ones = np.ones((1, LQ), dtype=np.float32)
    in_maps = []
    for b in range(B):
        in_maps.append({
            "qaug": np.concatenate([Q[b].T, ones], axis=0),
            "kaug": np.concatenate([K[b].T, ones], axis=0),
            "v": np.ascontiguousarray(V[b]).astype(ml_dtypes.bfloat16),
            "wqa": wqa, "wka": wka, "coef": coef,
        })
    return in_maps


def _get_nc():
    if "nc" not in _CACHE:
        _CACHE["nc"] = _build()
    return _CACHE["nc"]


def run(Q, K, V, W_q, W_k, v_w, trace=False, **spmd_kwargs):
    nc = _get_nc()
    in_maps = _prep_inputs(Q, K, V, W_q, W_k, v_w)
    res = run_bass_kernel_spmd(nc, in_maps, core_ids=list(range(N_CORES)),
                               trace=trace, **spmd_kwargs)
    out = np.stack([np.asarray(r["out"], dtype=np.float32) for r in res.results])
    attn = np.stack([np.asarray(r["attnT"].T, dtype=np.float32)
                     for r in res.results])
    return (out, attn), res


def kernel(Q, K, V, W_q, W_k, v_w):
    (out, attn), _ = run(Q, K, V, W_q, W_k, v_w)
    return (out, attn)


# revision 52
# speedup vs baseline: 1.0401x; 1.0338x over previous
"""Trainium2 Bass kernel for AdditiveAttention (B=8, Lq=Lk=512, d_k=64).

Data-parallel over batch across 8 NeuronCores; each core computes one batch
element entirely on-chip.

Math: scores[q,k] = sum_d v_w[d] * tanh(qp[q,d] + kp[k,d]) with
qp = Q W_q^T, kp = K W_k^T.  tanh(a+b) is expanded in a sine series
    tanh(s) ~= sum_m c_m sin(w_m s)   (R=12 terms, max err ~1.1e-3 on |s|<=13)
so with sin(w(a+b)) = sin(wa)cos(wb) + cos(wa)sin(wb) the whole energy
tensor collapses into a matmul over a 2*R*64 contraction:
    scores^T = F_k^T-chunks (stationary) @ F_q (moving), accumulated in PSUM.
Features are built on-device:
  - U = (w_m/2pi)*proj + b  via augmented-weight fp32 matmuls (PE)
  - exact range reduction: r = round(U) via the fp32 magic-number trick
    (ACT Identity adds 1.5*2^23, DVE scalar_tensor_tensor computes r - U)
  - sin via ACT Sin on the centered fraction (scale=-2pi), bf16 out
  - q-side scaled by c_m*v_w[d] (DVE per-partition tensor_scalar)
Softmax runs in transposed layout (exp on ScalarE straight from PSUM; row
sums via ones-matmul; reciprocal broadcast by PE outer product), attn is
written transposed (host returns a transposed view), and out = attn @ V uses
the exp^T tiles directly with a ones-column appended to V to recover the
softmax denominators in q-layout.

Self-contained: hardcodes shapes; host-side work is layout + tiny constants.
"""
import numpy as np
import ml_dtypes

import concourse.bass as bass
import concourse.tile as tile
from concourse import bacc, mybir
from concourse.bass_utils import run_bass_kernel_spmd

B = 8
LQ = 512
LK = 512
D = 64
N_CORES = 8

R = 12          # number of sine terms
FIT_S = 13.0    # fit domain half-width
FIT_W = 4.5     # max angular frequency (max err ~1.1e-3 on [-13, 13])
MAGIC = float(np.float32(1.5 * 2 ** 23))

FP32 = mybir.dt.float32
F32R = mybir.dt.float32r
BF16 = mybir.dt.bfloat16

_CACHE: dict = {}


def _fit_sines():
    w = (np.arange(R) + 0.5) * FIT_W / R
    s = np.linspace(-FIT_S, FIT_S, 8001)
    A = np.sin(np.outer(s, w))
    y = np.tanh(s)
    c = np.linalg.solve(A.T @ A + 1e-8 * np.eye(R), A.T @ y)
    return w.astype(np.float64), c.astype(np.float64)


FREQS, COEFS = _fit_sines()


def _build():
    nc = bacc.Bacc("TRN2", target_bir_lowering=False, debug=False,
                   num_devices=N_CORES)

    qaug = nc.declare_dram_parameter("qaug", [D + 1, LQ], F32R, isOutput=False)
    kaug = nc.declare_dram_parameter("kaug", [D + 1, LK], F32R, isOutput=False)
    v = nc.declare_dram_parameter("v", [LK, D], BF16, isOutput=False)
    wqa = nc.declare_dram_parameter("wqa", [D + 1, R * 128], F32R, isOutput=False)
    wka = nc.declare_dram_parameter("wka", [D + 1, R * 128], F32R, isOutput=False)
    coef = nc.declare_dram_parameter("coef", [128, R], FP32, isOutput=False)
    attnT_out = nc.declare_dram_parameter("attnT", [LK, LQ], BF16, isOutput=True)
    out_out = nc.declare_dram_parameter("out", [LQ, D], FP32, isOutput=True)

    with tile.TileContext(nc) as tc:
        _program(nc, tc, qaug, kaug, v, wqa, wka, coef, attnT_out, out_out)

    nc.compile()
    return nc


def _program(nc, tc, qaug, kaug, v, wqa, wka, coef, attnT_out, out_out):
    T1_ALT = True    # alternate the round-add between DVE and ScalarE
    NH = 1           # no q-split (PSUM accumulation is per-element via memset+start=False)
    COEF_GP = True   # coefficient scaling on GpSimd
    TAIL_GP = False
    HW = LQ // NH
    QTILES = LQ // 128  # 4
    KCH = LK // 128     # 4
    SINF = mybir.ActivationFunctionType.Sin
    IDF = mybir.ActivationFunctionType.Identity
    EXPF = mybir.ActivationFunctionType.Exp

    with (
        tc.tile_pool(name="const", bufs=1) as const_pool,
        tc.tile_pool(name="work", bufs=1) as work_pool,
        tc.tile_pool(name="scT_ps", bufs=1, space="PSUM") as scT_pool,
    ):
        # ---- constants / inputs ----
        qaug_sb = const_pool.tile([D + 1, LQ], F32R)
        nc.sync.dma_start(qaug_sb[:], qaug[:])
        kaug_sb = const_pool.tile([D + 1, LK], F32R)
        nc.sync.dma_start(kaug_sb[:], kaug[:])
        wqa_sb = const_pool.tile([D + 1, R * 128], F32R)
        wka_sb = const_pool.tile([D + 1, R * 128], F32R)
        warm_sb = const_pool.tile([128, 128], BF16)
        nc.vector.memset(warm_sb[:, 0:16], 0.0)
        with tc.tile_pool(name="warm_ps", bufs=1, space="PSUM") as warm_pool:
            warm_ps = warm_pool.tile([128, 128], FP32)
            for _ in range(10):
                nc.tensor.matmul(warm_ps[:], warm_sb[:], warm_sb[:],
                                 start=True, stop=True)
        first = 2 * 128
        nc.sync.dma_start(wqa_sb[:, 0:first], wqa[:, 0:first])
        nc.sync.dma_start(wka_sb[:, 0:first], wka[:, 0:first])
        nc.sync.dma_start(wqa_sb[:, first:R * 128], wqa[:, first:R * 128])
        nc.sync.dma_start(wka_sb[:, first:R * 128], wka[:, first:R * 128])
        coef_sb = const_pool.tile([128, R], FP32)
        nc.sync.dma_start(coef_sb[:], coef[:])
        magic = const_pool.tile([128, 1], FP32)
        nc.vector.memset(magic[:], MAGIC)
        ones_col = const_pool.tile([128, 1], BF16)
        nc.vector.memset(ones_col[:], 1.0)
        ones_row = const_pool.tile([1, 128], BF16)
        nc.vector.memset(ones_row[:], 1.0)
        v1_tiles = const_pool.tile([128, KCH * (D + 1)], BF16)
        v_r = v.rearrange("(c p) d -> p c d", p=128)
        v1_r = v1_tiles.rearrange("p (c e) -> p c e", c=KCH)
        nc.sync.dma_start(v1_r[:, :, 0:D], v_r[:])
        for c in range(KCH):
            nc.vector.memset(v1_tiles[:, c * (D + 1) + D: (c + 1) * (D + 1)], 1.0)

        # ---- phase 1+2: features and score matmuls, pipelined over i ----
        # interleaved layout: block i = [q-side freq i (512) | k-side freq i (512)]
        w_all = work_pool.tile([128, R * 1024], FP32)  # centered -frac(U)
        f_all = work_pool.tile([128, R * 1024], BF16)  # sin features (raw)
        fq = work_pool.tile([128, R * LQ], BF16)       # coef-scaled q features
        scT = [scT_pool.tile([128, LQ], FP32, name=f"scT{c}") for c in range(KCH)]

        # software-pipelined over i: stage deps are >=1 iteration old so no
        # engine queue ever stalls on an in-flight producer.
        with tc.tile_pool(name="u_ps", bufs=2, space="PSUM") as u_pool:
          u_tiles = {}
          t1_tiles = {}
          for it in range(R + 2):
            if it < R:
                i = it
                u_ps = u_pool.tile([128, 1024], FP32, name="u_ps", tag="u_ps")
                u_tiles[i] = u_ps
                nc.tensor.matmul(u_ps[:, 0:512], wqa_sb[:, i * 128:(i + 1) * 128],
                                 qaug_sb[:], start=True, stop=True)
                nc.tensor.matmul(u_ps[:, 512:1024], wka_sb[:, i * 128:(i + 1) * 128],
                                 kaug_sb[:], start=True, stop=True)
                if i >= 4:
                    # full magic-number round for |u| that may exceed 1.5
                    t1 = work_pool.tile([128, 1024], FP32, name="t1", tag="t1",
                                        bufs=4)
                    t1_tiles[i] = t1
                    if T1_ALT and i in (4, 6, 8):
                        nc.vector.tensor_scalar(t1[:], u_ps[:], MAGIC, None,
                                                mybir.AluOpType.add)
                    else:
                        nc.scalar.activation(t1[:], u_ps[:], IDF,
                                             bias=magic[:, 0:1])
            if 1 <= it <= R:
                i = it - 1
                if i >= 4:
                    nc.vector.scalar_tensor_tensor(
                        w_all[:, i * 1024:(i + 1) * 1024], t1_tiles.pop(i)[:],
                        -MAGIC, u_tiles.pop(i)[:],
                        mybir.AluOpType.add, mybir.AluOpType.subtract)
                elif i >= 1:
                    # 0.5 < |u| <= 1.45: at most one period off -- single
                    # conditional wrap into [-0.5, 0.5]
                    nc.vector.add_range_wrap(
                        w_all[:, i * 1024:(i + 1) * 1024], u_tiles.pop(i)[:],
                        shift=0.0, bound=0.5, period=1.0)
            if 2 <= it:
                i = it - 2
                lo, hi = i * 1024, (i + 1) * 1024
                if i == 0:
                    nc.scalar.activation(f_all[:, lo:hi], u_tiles.pop(0)[:], SINF,
                                         scale=float(2 * np.pi))
                elif i < 4:
                    # wrap path yields +frac-centered values
                    nc.scalar.activation(f_all[:, lo:hi], w_all[:, lo:hi], SINF,
                                         scale=float(2 * np.pi))
                else:
                    nc.scalar.activation(f_all[:, lo:hi], w_all[:, lo:hi], SINF,
                                         scale=float(-2 * np.pi))
                _coef_eng = nc.gpsimd if COEF_GP else nc.vector
                _coef_eng.tensor_scalar(
                    fq[:, i * 512:(i + 1) * 512],
                    f_all[:, i * 1024: i * 1024 + 512],
                    coef_sb[:, i:i + 1], None, mybir.AluOpType.mult)
                for c in range(KCH):
                    for h in range(NH):
                        nc.tensor.matmul(
                            scT[c][:, h * HW: (h + 1) * HW],
                            f_all[:, i * 1024 + 512 + c * 128:
                                  i * 1024 + 512 + (c + 1) * 128],
                            fq[:, i * 512 + h * HW: i * 512 + (h + 1) * HW],
                            start=(i == 0), stop=(i == R - 1))

        # ---- phase 3+4 per q-half: softmax in T layout, attnT out, AV ----
        expT = work_pool.tile([128, KCH * LQ], BF16)
        with (
            tc.tile_pool(name="sm_ps", bufs=1, space="PSUM") as sm_ps,
            tc.tile_pool(name="av_ps", bufs=2, space="PSUM") as av_ps_pool,
        ):
            for h in range(NH):
                hlo = h * HW
                for c in range(KCH):
                    nc.scalar.activation(
                        expT[:, c * LQ + hlo: c * LQ + hlo + HW],
                        scT[c][:, hlo: hlo + HW], EXPF)
                sums_row = sm_ps.tile([1, HW], FP32, name="sums_row", tag="sums")
                for c in range(KCH):
                    nc.tensor.matmul(sums_row[:], ones_col[:],
                                     expT[:, c * LQ + hlo: c * LQ + hlo + HW],
                                     start=(c == 0), stop=(c == KCH - 1))
                recip_row = work_pool.tile([1, HW], BF16, name="recip_row",
                                           tag="recip_row", bufs=NH)
                with nc.allow_low_precision(reason="bf16 softmax recip bcast"):
                    nc.vector.reciprocal(recip_row[:], sums_row[:])
                rbc_ps = sm_ps.tile([128, HW], FP32, name="rbc_ps", tag="rbc")
                nc.tensor.matmul(rbc_ps[:], ones_row[:], recip_row[:],
                                 start=True, stop=True)
                rbc_sb = work_pool.tile([128, HW], BF16, name="rbc_sb",
                                        tag="rbc_sb", bufs=NH)
                nc.scalar.activation(rbc_sb[:], rbc_ps[:], IDF)
                for c in range(KCH):
                    a_t = work_pool.tile([128, HW], BF16, name="a_t", tag="a_t",
                                         bufs=4)
                    eng = nc.gpsimd if (TAIL_GP and c % 2 == 1) else nc.vector
                    eng.tensor_tensor(a_t[:],
                                      expT[:, c * LQ + hlo: c * LQ + hlo + HW],
                                      rbc_sb[:], mybir.AluOpType.mult)
                    nc.sync.dma_start(attnT_out[c * 128:(c + 1) * 128, hlo:hlo + HW],
                                      a_t[:])
                o_all = work_pool.tile([128, (HW // 128) * D], FP32,
                                       name="o_all", tag="o_all", bufs=NH)
                for t in range(HW // 128):
                    qlo = hlo + t * 128
                    av_ps = av_ps_pool.tile([128, D + 1], FP32, name="av_ps")
                    for c in range(KCH):
                        nc.tensor.matmul(av_ps[:],
                                         expT[:, c * LQ + qlo: c * LQ + qlo + 128],
                                         v1_tiles[:, c * (D + 1):(c + 1) * (D + 1)],
                                         start=(c == 0), stop=(c == KCH - 1))
                    recip_q = work_pool.tile([128, 1], FP32, name="recip_q",
                                             tag="recip_q", bufs=2)
                    nc.vector.reciprocal(recip_q[:], av_ps[:, D:D + 1])
                    nc.vector.tensor_scalar(o_all[:, t * D:(t + 1) * D],
                                            av_ps[:, 0:D], recip_q[:, 0:1],
                                            None, mybir.AluOpType.mult)
                out_r = out_out.rearrange("(t p) d -> p t d", p=128)
                o_all_r = o_all.rearrange("p (t d) -> p t d", d=D)
                nc.sync.dma_start(out_r[:], o_all_r[:])


def _prep_inputs(Q, K, V, W_q, W_k, v_w):
    Q = np.asarray(Q, dtype=np.float32)
    K = np.asarray(K, dtype=np.float32)
    V = np.asarray(V, dtype=np.float32)
    W_q = np.asarray(W_q, dtype=np.float32)
    W_k = np.asarray(W_k, dtype=np.float32)
    v_w = np.asarray(v_w, dtype=np.float32)

    f = (FREQS / (2 * np.pi)).astype(np.float64)
    # wqa tile i, col p: rows 0-63 = f_i * W_q[p%64, :]; row 64 = b (0 sin / .25 cos)
    wqa = np.zeros((D + 1, R * 128), dtype=np.float32)
    wka = np.zeros((D + 1, R * 128), dtype=np.float32)
    for i in range(R):
        blk_q = np.concatenate([f[i] * W_q.T, f[i] * W_q.T], axis=1)  # [64, 128]
        blk_k = np.concatenate([f[i] * W_k.T, f[i] * W_k.T], axis=1)
        wqa[:D, i * 128:(i + 1) * 128] = blk_q
        wka[:D, i * 128:(i + 1) * 128] = blk_k
        # q side: rows 0-63 sin (b=0), 64-127 cos (b=0.25)
        wqa[D, i * 128 + 64: (i + 1) * 128] = 0.25
        # k side: rows 0-63 cos (b=0.25), 64-127 sin (b=0)
        wka[D, i * 128: i * 128 + 64] = 0.25
    coef = np.zeros((128, R), dtype=np.float32)
    for i in range(R):
        cw = (COEFS[i] * v_w.astype(np.float64)).astype(np.float32)
        coef[:D, i] = cw
        coef[D:, i] = cw

    ones = np.ones((1, LQ), dtype=np.float32)
    in_maps = []
    for b in range(B):
        in_maps.append({
            "qaug": np.concatenate([Q[b].T, ones], axis=0),
            "kaug": np.concatenate([K[b].T, ones], axis=0),
            "v": np.ascontiguousarray(V[b]).astype(ml_dtypes.bfloat16),
            "wqa": wqa, "wka": wka, "coef": coef,
        })
    return in_maps


def _get_nc():
    if "nc" not in _CACHE:
        _CACHE["nc"] = _build()
    return _CACHE["nc"]


def run(Q, K, V, W_q, W_k, v_w, trace=False, **spmd_kwargs):
    nc = _get_nc()
    in_maps = _prep_inputs(Q, K, V, W_q, W_k, v_w)
    res = run_bass_kernel_spmd(nc, in_maps, core_ids=list(range(N_CORES)),
                               trace=trace, **spmd_kwargs)
    out = np.stack([np.asarray(r["out"], dtype=np.float32) for r in res.results])
    attn = np.stack([np.asarray(r["attnT"].T, dtype=np.float32)
                     for r in res.results])
    return (out, attn), res


def kernel(Q, K, V, W_q, W_k, v_w):
    (out, attn), _ = run(Q, K, V, W_q, W_k, v_w)
    return (out, attn)


# revision 53
# speedup vs baseline: 1.0470x; 1.0066x over previous
"""Trainium2 Bass kernel for AdditiveAttention (B=8, Lq=Lk=512, d_k=64).

Data-parallel over batch across 8 NeuronCores; each core computes one batch
element entirely on-chip.

Math: scores[q,k] = sum_d v_w[d] * tanh(qp[q,d] + kp[k,d]) with
qp = Q W_q^T, kp = K W_k^T.  tanh(a+b) is expanded in a sine series
    tanh(s) ~= sum_m c_m sin(w_m s)   (R=12 terms, max err ~1.1e-3 on |s|<=13)
so with sin(w(a+b)) = sin(wa)cos(wb) + cos(wa)sin(wb) the whole energy
tensor collapses into a matmul over a 2*R*64 contraction:
    scores^T = F_k^T-chunks (stationary) @ F_q (moving), accumulated in PSUM.
Features are built on-device:
  - U = (w_m/2pi)*proj + b  via augmented-weight fp32 matmuls (PE)
  - exact range reduction: r = round(U) via the fp32 magic-number trick
    (ACT Identity adds 1.5*2^23, DVE scalar_tensor_tensor computes r - U)
  - sin via ACT Sin on the centered fraction (scale=-2pi), bf16 out
  - q-side scaled by c_m*v_w[d] (DVE per-partition tensor_scalar)
Softmax runs in transposed layout (exp on ScalarE straight from PSUM; row
sums via ones-matmul; reciprocal broadcast by PE outer product), attn is
written transposed (host returns a transposed view), and out = attn @ V uses
the exp^T tiles directly with a ones-column appended to V to recover the
softmax denominators in q-layout.

Self-contained: hardcodes shapes; host-side work is layout + tiny constants.
"""
import numpy as np
import ml_dtypes

import concourse.bass as bass
import concourse.tile as tile
from concourse import bacc, mybir
from concourse.bass_utils import run_bass_kernel_spmd

B = 8
LQ = 512
LK = 512
D = 64
N_CORES = 8

R = 12          # number of sine terms
FIT_S = 13.0    # fit domain half-width
FIT_W = 4.5     # max angular frequency (max err ~1.1e-3 on [-13, 13])
MAGIC = float(np.float32(1.5 * 2 ** 23))

FP32 = mybir.dt.float32
F32R = mybir.dt.float32r
BF16 = mybir.dt.bfloat16

_CACHE: dict = {}


def _fit_sines():
    w = (np.arange(R) + 0.5) * FIT_W / R
    s = np.linspace(-FIT_S, FIT_S, 8001)
    A = np.sin(np.outer(s, w))
    y = np.tanh(s)
    c = np.linalg.solve(A.T @ A + 1e-8 * np.eye(R), A.T @ y)
    return w.astype(np.float64), c.astype(np.float64)


FREQS, COEFS = _fit_sines()


def _build():
    nc = bacc.Bacc("TRN2", target_bir_lowering=False, debug=False,
                   num_devices=N_CORES)

    qaug = nc.declare_dram_parameter("qaug", [D + 1, LQ], F32R, isOutput=False)
    kaug = nc.declare_dram_parameter("kaug", [D + 1, LK], F32R, isOutput=False)
    v = nc.declare_dram_parameter("v", [LK, D], BF16, isOutput=False)
    wqa = nc.declare_dram_parameter("wqa", [D + 1, R * 128], F32R, isOutput=False)
    wka = nc.declare_dram_parameter("wka", [D + 1, R * 128], F32R, isOutput=False)
    coef = nc.declare_dram_parameter("coef", [128, R], FP32, isOutput=False)
    attnT_out = nc.declare_dram_parameter("attnT", [LK, LQ], BF16, isOutput=True)
    out_out = nc.declare_dram_parameter("out", [LQ, D], FP32, isOutput=True)

    with tile.TileContext(nc) as tc:
        _program(nc, tc, qaug, kaug, v, wqa, wka, coef, attnT_out, out_out)

    nc.compile()
    return nc


def _program(nc, tc, qaug, kaug, v, wqa, wka, coef, attnT_out, out_out):
    T1_ALT = True    # alternate the round-add between DVE and ScalarE
    NH = 1           # no q-split (PSUM accumulation is per-element via memset+start=False)
    COEF_GP = True   # coefficient scaling on GpSimd
    TAIL_GP = False
    HW = LQ // NH
    QTILES = LQ // 128  # 4
    KCH = LK // 128     # 4
    SINF = mybir.ActivationFunctionType.Sin
    IDF = mybir.ActivationFunctionType.Identity
    EXPF = mybir.ActivationFunctionType.Exp

    with (
        tc.tile_pool(name="const", bufs=1) as const_pool,
        tc.tile_pool(name="work", bufs=1) as work_pool,
        tc.tile_pool(name="scT_ps", bufs=1, space="PSUM") as scT_pool,
    ):
        # ---- constants / inputs ----
        qaug_sb = const_pool.tile([D + 1, LQ], F32R)
        nc.sync.dma_start(qaug_sb[:], qaug[:])
        kaug_sb = const_pool.tile([D + 1, LK], F32R)
        nc.sync.dma_start(kaug_sb[:], kaug[:])
        wqa_sb = const_pool.tile([D + 1, R * 128], F32R)
        wka_sb = const_pool.tile([D + 1, R * 128], F32R)
        warm_sb = const_pool.tile([128, 128], BF16)
        nc.vector.memset(warm_sb[:, 0:16], 0.0)
        with tc.tile_pool(name="warm_ps", bufs=1, space="PSUM") as warm_pool:
            warm_ps = warm_pool.tile([128, 128], FP32)
            for _ in range(10):
                nc.tensor.matmul(warm_ps[:], warm_sb[:], warm_sb[:],
                                 start=True, stop=True)
        first = 2 * 128
        nc.sync.dma_start(wqa_sb[:, 0:first], wqa[:, 0:first])
        nc.sync.dma_start(wka_sb[:, 0:first], wka[:, 0:first])
        nc.sync.dma_start(wqa_sb[:, first:R * 128], wqa[:, first:R * 128])
        nc.sync.dma_start(wka_sb[:, first:R * 128], wka[:, first:R * 128])
        coef_sb = const_pool.tile([128, R], FP32)
        nc.sync.dma_start(coef_sb[:], coef[:])
        magic = const_pool.tile([128, 1], FP32)
        nc.vector.memset(magic[:], MAGIC)
        ones_col = const_pool.tile([128, 1], BF16)
        nc.vector.memset(ones_col[:], 1.0)
        ones_row = const_pool.tile([1, 128], BF16)
        nc.vector.memset(ones_row[:], 1.0)
        v1_tiles = const_pool.tile([128, KCH * (D + 1)], BF16)
        v_r = v.rearrange("(c p) d -> p c d", p=128)
        v1_r = v1_tiles.rearrange("p (c e) -> p c e", c=KCH)
        nc.sync.dma_start(v1_r[:, :, 0:D], v_r[:])
        for c in range(KCH):
            nc.vector.memset(v1_tiles[:, c * (D + 1) + D: (c + 1) * (D + 1)], 1.0)

        # ---- phase 1+2: features and score matmuls, pipelined over i ----
        # interleaved layout: block i = [q-side freq i (512) | k-side freq i (512)]
        w_all = work_pool.tile([128, R * 1024], FP32)  # centered -frac(U)
        f_all = work_pool.tile([128, R * 1024], BF16)  # sin features (raw)
        fq = work_pool.tile([128, R * LQ], BF16)       # coef-scaled q features
        scT = [scT_pool.tile([128, LQ], FP32, name=f"scT{c}") for c in range(KCH)]

        # software-pipelined over i: stage deps are >=1 iteration old so no
        # engine queue ever stalls on an in-flight producer.
        with tc.tile_pool(name="u_ps", bufs=2, space="PSUM") as u_pool:
          u_tiles = {}
          t1_tiles = {}
          for it in range(R + 2):
            if it < R:
                i = it
                u_ps = u_pool.tile([128, 1024], FP32, name="u_ps", tag="u_ps")
                u_tiles[i] = u_ps
                nc.tensor.matmul(u_ps[:, 0:512], wqa_sb[:, i * 128:(i + 1) * 128],
                                 qaug_sb[:], start=True, stop=True)
                nc.tensor.matmul(u_ps[:, 512:1024], wka_sb[:, i * 128:(i + 1) * 128],
                                 kaug_sb[:], start=True, stop=True)
                if i >= 4:
                    # full magic-number round for |u| that may exceed 1.5
                    t1 = work_pool.tile([128, 1024], FP32, name="t1", tag="t1",
                                        bufs=4)
                    t1_tiles[i] = t1
                    if T1_ALT and i in (4, 6, 8, 10):
                        nc.vector.tensor_scalar(t1[:], u_ps[:], MAGIC, None,
                                                mybir.AluOpType.add)
                    else:
                        nc.scalar.activation(t1[:], u_ps[:], IDF,
                                             bias=magic[:, 0:1])
            if 1 <= it <= R:
                i = it - 1
                if i >= 4:
                    nc.vector.scalar_tensor_tensor(
                        w_all[:, i * 1024:(i + 1) * 1024], t1_tiles.pop(i)[:],
                        -MAGIC, u_tiles.pop(i)[:],
                        mybir.AluOpType.add, mybir.AluOpType.subtract)
                elif i >= 1:
                    # 0.5 < |u| <= 1.45: at most one period off -- single
                    # conditional wrap into [-0.5, 0.5]
                    nc.vector.add_range_wrap(
                        w_all[:, i * 1024:(i + 1) * 1024], u_tiles.pop(i)[:],
                        shift=0.0, bound=0.5, period=1.0)
            if 2 <= it:
                i = it - 2
                lo, hi = i * 1024, (i + 1) * 1024
                if i == 0:
                    nc.scalar.activation(f_all[:, lo:hi], u_tiles.pop(0)[:], SINF,
                                         scale=float(2 * np.pi))
                elif i < 4:
                    # wrap path yields +frac-centered values
                    nc.scalar.activation(f_all[:, lo:hi], w_all[:, lo:hi], SINF,
                                         scale=float(2 * np.pi))
                else:
                    nc.scalar.activation(f_all[:, lo:hi], w_all[:, lo:hi], SINF,
                                         scale=float(-2 * np.pi))
                _coef_eng = nc.gpsimd if COEF_GP else nc.vector
                _coef_eng.tensor_scalar(
                    fq[:, i * 512:(i + 1) * 512],
                    f_all[:, i * 1024: i * 1024 + 512],
                    coef_sb[:, i:i + 1], None, mybir.AluOpType.mult)
                for c in range(KCH):
                    for h in range(NH):
                        nc.tensor.matmul(
                            scT[c][:, h * HW: (h + 1) * HW],
                            f_all[:, i * 1024 + 512 + c * 128:
                                  i * 1024 + 512 + (c + 1) * 128],
                            fq[:, i * 512 + h * HW: i * 512 + (h + 1) * HW],
                            start=(i == 0), stop=(i == R - 1))

        # ---- phase 3+4 per q-half: softmax in T layout, attnT out, AV ----
        expT = work_pool.tile([128, KCH * LQ], BF16)
        with (
            tc.tile_pool(name="sm_ps", bufs=1, space="PSUM") as sm_ps,
            tc.tile_pool(name="av_ps", bufs=2, space="PSUM") as av_ps_pool,
        ):
            for h in range(NH):
                hlo = h * HW
                for c in range(KCH):
                    nc.scalar.activation(
                        expT[:, c * LQ + hlo: c * LQ + hlo + HW],
                        scT[c][:, hlo: hlo + HW], EXPF)
                sums_row = sm_ps.tile([1, HW], FP32, name="sums_row", tag="sums")
                for c in range(KCH):
                    nc.tensor.matmul(sums_row[:], ones_col[:],
                                     expT[:, c * LQ + hlo: c * LQ + hlo + HW],
                                     start=(c == 0), stop=(c == KCH - 1))
                recip_row = work_pool.tile([1, HW], BF16, name="recip_row",
                                           tag="recip_row", bufs=NH)
                with nc.allow_low_precision(reason="bf16 softmax recip bcast"):
                    nc.vector.reciprocal(recip_row[:], sums_row[:])
                rbc_ps = sm_ps.tile([128, HW], FP32, name="rbc_ps", tag="rbc")
                nc.tensor.matmul(rbc_ps[:], ones_row[:], recip_row[:],
                                 start=True, stop=True)
                rbc_sb = work_pool.tile([128, HW], BF16, name="rbc_sb",
                                        tag="rbc_sb", bufs=NH)
                nc.scalar.activation(rbc_sb[:], rbc_ps[:], IDF)
                for c in range(KCH):
                    a_t = work_pool.tile([128, HW], BF16, name="a_t", tag="a_t",
                                         bufs=4)
                    eng = nc.gpsimd if (TAIL_GP and c % 2 == 1) else nc.vector
                    eng.tensor_tensor(a_t[:],
                                      expT[:, c * LQ + hlo: c * LQ + hlo + HW],
                                      rbc_sb[:], mybir.AluOpType.mult)
                    nc.sync.dma_start(attnT_out[c * 128:(c + 1) * 128, hlo:hlo + HW],
                                      a_t[:])
                o_all = work_pool.tile([128, (HW // 128) * D], FP32,
                                       name="o_all", tag="o_all", bufs=NH)
                for t in range(HW // 128):
                    qlo = hlo + t * 128
                    av_ps = av_ps_pool.tile([128, D + 1], FP32, name="av_ps")
                    for c in range(KCH):
                        nc.tensor.matmul(av_ps[:],
                                         expT[:, c * LQ + qlo: c * LQ + qlo + 128],
                                         v1_tiles[:, c * (D + 1):(c + 1) * (D + 1)],
                                         start=(c == 0), stop=(c == KCH - 1))
                    recip_q = work_pool.tile([128, 1], FP32, name="recip_q",
                                             tag="recip_q", bufs=2)
                    nc.vector.reciprocal(recip_q[:], av_ps[:, D:D + 1])
                    nc.vector.tensor_scalar(o_all[:, t * D:(t + 1) * D],
                                            av_ps[:, 0:D], recip_q[:, 0:1],
                                            None, mybir.AluOpType.mult)
                out_r = out_out.rearrange("(t p) d -> p t d", p=128)
                o_all_r = o_all.rearrange("p (t d) -> p t d", d=D)
                nc.sync.dma_start(out_r[:], o_all_r[:])


def _prep_inputs(Q, K, V, W_q, W_k, v_w):
    Q = np.asarray(Q, dtype=np.float32)
    K = np.asarray(K, dtype=np.float32)
    V = np.asarray(V, dtype=np.float32)
    W_q = np.asarray(W_q, dtype=np.float32)
    W_k = np.asarray(W_k, dtype=np.float32)
    v_w = np.asarray(v_w, dtype=np.float32)

    f = (FREQS / (2 * np.pi)).astype(np.float64)
    # wqa tile i, col p: rows 0-63 = f_i * W_q[p%64, :]; row 64 = b (0 sin / .25 cos)
    wqa = np.zeros((D + 1, R * 128), dtype=np.float32)
    wka = np.zeros((D + 1, R * 128), dtype=np.float32)
    for i in range(R):
        blk_q = np.concatenate([f[i] * W_q.T, f[i] * W_q.T], axis=1)  # [64, 128]
        blk_k = np.concatenate([f[i] * W_k.T, f[i] * W_k.T], axis=1)
        wqa[:D, i * 128:(i + 1) * 128] = blk_q
        wka[:D, i * 128:(i + 1) * 128] = blk_k
        # q side: rows 0-63 sin (b=0), 64-127 cos (b=0.25)
        wqa[D, i * 128 + 64: (i + 1) * 128] = 0.25
        # k side: rows 0-63 cos (b=0.25), 64-127 sin (b=0)
        wka[D, i * 128: i * 128 + 64] = 0.25
    coef = np.zeros((128, R), dtype=np.float32)
    for i in range(R):
        cw = (COEFS[i] * v_w.astype(np.float64)).astype(np.float32)
        coef[:D, i] = cw
        coef[D:, i] = cw

    ones = np.ones((1, LQ), dtype=np.float32)
    in_maps = []
    for b in range(B):
        in_maps.append({
            "qaug": np.concatenate([Q[b].T, ones], axis=0),
            "kaug": np.concatenate([K[b].T, ones], axis=0),
            "v": np.ascontiguousarray(V[b]).astype(ml_dtypes.bfloat16),
            "wqa": wqa, "wka": wka, "coef": coef,
        })
    return in_maps


def _get_nc():
    if "nc" not in _CACHE:
        _CACHE["nc"] = _build()
    return _CACHE["nc"]


def run(Q, K, V, W_q, W_k, v_w, trace=False, **spmd_kwargs):
    nc = _get_nc()
    in_maps = _prep_inputs(Q, K, V, W_q, W_k, v_w)
    res = run_bass_kernel_spmd(nc, in_maps, core_ids=list(range(N_CORES)),
                               trace=trace, **spmd_kwargs)
    out = np.stack([np.asarray(r["out"], dtype=np.float32) for r in res.results])
    attn = np.stack([np.asarray(r["attnT"].T, dtype=np.float32)
                     for r in res.results])
    return (out, attn), res


def kernel(Q, K, V, W_q, W_k, v_w):
    (out, attn), _ = run(Q, K, V, W_q, W_k, v_w)
    return (out, attn)
